# revision 3
# baseline (speedup 1.0000x reference)
import sys, os, time
sys.path.insert(0, '/opt/trn_rl_repo')
os.environ.setdefault("JAX_PLATFORMS", "")

import numpy as np
import ml_dtypes

import concourse.bass as bass
import concourse.bacc as bacc
import concourse.mybir as mybir
import concourse.tile as tile
from concourse.bass_utils import run_bass_kernel_spmd

BF = ml_dtypes.bfloat16
B, S, D, H, DH = 4, 2048, 2048, 16, 128
INV_SQRT_DH = 1.0 / np.sqrt(128.0)

_CACHE = {}
LAST_EXEC_NS = None


def _build():
    nc = bacc.Bacc("TRN2", target_bir_lowering=False, debug=False, num_devices=8)
    f32, bf16 = mybir.dt.float32, mybir.dt.bfloat16
    x_ap = nc.dram_tensor("x_img", (128, 32768), bf16, kind="ExternalInput").ap()
    wqk_ap = nc.dram_tensor("wqk_img", (128, 32768), bf16, kind="ExternalInput").ap()
    wv_ap = nc.dram_tensor("wv_img", (128, 16384), bf16, kind="ExternalInput").ap()
    wout_ap = nc.dram_tensor("wout_img", (128, 16384), bf16, kind="ExternalInput").ap()
    bqk_ap = nc.dram_tensor("bqk", (128, 16), f32, kind="ExternalInput").ap()
    bv_ap = nc.dram_tensor("bias_v", (128, 1024), f32, kind="ExternalInput").ap()
    mask_ap = nc.dram_tensor("masks", (128, 2048), f32, kind="ExternalInput").ap()
    onc_ap = nc.dram_tensor("ones_col", (128, 1), bf16, kind="ExternalInput").ap()
    onr_ap = nc.dram_tensor("ones_row", (1, 128), f32, kind="ExternalInput").ap()
    out_ap = nc.dram_tensor("out", (2048, 2048), f32, kind="ExternalOutput").ap()

    with tile.TileContext(nc) as tc:
        with tc.tile_pool(name="persist", bufs=1) as pp:
            qk_sb = pp.tile([128, 32768], bf16)    # [dh, ob*2048+s], ob=2hl: Q_h^T, 2hl+1: K_h^T
            v_sb = pp.tile([128, 16384], bf16)     # [s%128, st*1024 + hl*128 + dh]
            bqk_sb = pp.tile([128, 16], f32)
            bv_sb = pp.tile([128, 1024], f32)
            ones_c = pp.tile([128, 1], bf16)
            ones_r = pp.tile([1, 128], f32)
            nc.sync.dma_start(bqk_sb, bqk_ap)
            nc.sync.dma_start(bv_sb, bv_ap)
            nc.sync.dma_start(ones_c, onc_ap)
            nc.sync.dma_start(ones_r, onr_ap)

            # ---------------- Stage A: QKV projections ----------------
            with tc.tile_pool(name="xTp", bufs=1) as xTp:
                xT = xTp.tile([128, 32768], bf16)  # [d%128, dt*2048 + s]
                for dt in range(16):
                    nc.sync.dma_start(xT[:, dt*2048:(dt+1)*2048],
                                      x_ap[:, dt*2048:(dt+1)*2048])

                # A-V first: v_sb = x @ Wv^T + bv   (psum[s, ov])
                with (
                    tc.tile_pool(name="wvp", bufs=1) as wvp,
                    tc.tile_pool(name="psV", bufs=4, space="PSUM") as psV,
                ):
                    for oc in range(2):
                        wv_oc = wvp.tile([128, 8192], bf16)  # [d%128, dt*512 + ov%512]
                        for dt in range(16):
                            nc.sync.dma_start(
                                wv_oc[:, dt*512:(dt+1)*512],
                                wv_ap[:, dt*1024 + oc*512: dt*1024 + (oc+1)*512])
                        for st in range(16):
                            ps = psV.tile([128, 512], f32)
                            for dt in range(16):
                                nc.tensor.matmul(
                                    ps,
                                    xT[:, dt*2048 + st*128: dt*2048 + (st+1)*128],
                                    wv_oc[:, dt*512:(dt+1)*512],
                                    start=(dt == 0), stop=(dt == 15))
                            nc.vector.tensor_tensor(
                                v_sb[:, st*1024 + oc*512: st*1024 + (oc+1)*512],
                                ps, bv_sb[:, oc*512:(oc+1)*512], mybir.AluOpType.add)

                # A-QK: qk_sb = Wqk @ x^T + b      (psum[o, s])
                with (
                    tc.tile_pool(name="wqkp", bufs=2) as wqkp,
                    tc.tile_pool(name="psQ", bufs=4, space="PSUM") as psQ,
                ):
                    for ob in range(16):
                        slab = wqkp.tile([128, 2048], bf16)  # [d%128, dt*128 + o]
                        nc.sync.dma_start(slab, wqk_ap[:, ob*2048:(ob+1)*2048])
                        for sc in range(4):
                            ps = psQ.tile([128, 512], f32)
                            for dt in range(16):
                                nc.tensor.matmul(
                                    ps,
                                    slab[:, dt*128:(dt+1)*128],
                                    xT[:, dt*2048 + sc*512: dt*2048 + (sc+1)*512],
                                    start=(dt == 0), stop=(dt == 15))
                            nc.vector.tensor_scalar_add(
                                qk_sb[:, ob*2048 + sc*512: ob*2048 + (sc+1)*512],
                                ps, bqk_sb[:, ob:ob+1])

            # ---------------- Stage B: attention ----------------
            with tc.tile_pool(name="bstat", bufs=1) as bstat:
                masks = bstat.tile([128, 2048], f32)
                ctxT = bstat.tile([128, 16384], bf16)  # [dh, hl*2048 + q]
                wout = bstat.tile([128, 16384], bf16)  # [dh, hl*2048 + od]
                nc.sync.dma_start(masks, mask_ap)
                for i in range(8):
                    nc.sync.dma_start(wout[:, i*2048:(i+1)*2048],
                                      wout_ap[:, i*2048:(i+1)*2048])

                with (
                    tc.tile_pool(name="expp", bufs=4) as expp,
                    tc.tile_pool(name="accp", bufs=2) as accp,
                    tc.tile_pool(name="accbfp", bufs=2) as accbfp,
                    tc.tile_pool(name="recp", bufs=2) as recp,
                    tc.tile_pool(name="bcp", bufs=2) as bcp,
                    tc.tile_pool(name="psS", bufs=4, space="PSUM") as psS,
                    tc.tile_pool(name="psC", bufs=2, space="PSUM") as psC,
                    tc.tile_pool(name="psD", bufs=1, space="PSUM") as psD,
                    tc.tile_pool(name="psB", bufs=1, space="PSUM") as psB,
                ):
                    for hl in range(8):
                        qb = (2*hl) * 2048
                        kb = (2*hl+1) * 2048
                        for ic in range(4):
                            njt = 4*ic + 4
                            acc = accp.tile([128, 512], f32)
                            acc_bf = accbfp.tile([128, 512], bf16)
                            cp = psC.tile([128, 512], f32)
                            pend = []

                            def flush_ctx():
                                jt_, ex_ = pend.pop(0)
                                nc.tensor.matmul(
                                    cp, v_sb[:, jt_*1024 + hl*128: jt_*1024 + (hl+1)*128],
                                    ex_, start=(jt_ == 0), stop=(jt_ == njt-1),
                                    skip_group_check=True)

                            for jt in range(njt):
                                sc = psS.tile([128, 512], f32)
                                nc.tensor.matmul(
                                    sc, qk_sb[:, kb + jt*128: kb + (jt+1)*128],
                                    qk_sb[:, qb + ic*512: qb + (ic+1)*512],
                                    start=True, stop=True, skip_group_check=True)
                                t = jt - 4*ic
                                if t >= 0:
                                    nc.vector.tensor_tensor(
                                        sc, sc, masks[:, t*512:(t+1)*512],
                                        mybir.AluOpType.add)
                                ex = expp.tile([128, 512], bf16)
                                nc.scalar.activation(ex, sc,
                                                     mybir.ActivationFunctionType.Exp)
                                if jt == 0:
                                    nc.vector.tensor_copy(acc, ex)
                                elif jt < njt - 1:
                                    nc.vector.tensor_tensor(acc, acc, ex,
                                                            mybir.AluOpType.add)
                                else:
                                    nc.vector.tensor_tensor(acc_bf, acc, ex,
                                                            mybir.AluOpType.add)
                                pend.append((jt, ex))
                                if len(pend) > 2:
                                    flush_ctx()
                            while pend:
                                flush_ctx()

                            dn = psD.tile([1, 512], f32)
                            nc.tensor.matmul(dn, ones_c, acc_bf, start=True, stop=True,
                                             skip_group_check=True)
                            rec = recp.tile([1, 512], f32)
                            nc.vector.reciprocal(rec, dn)
                            bc = psB.tile([128, 512], f32)
                            nc.tensor.matmul(bc, ones_r, rec, start=True, stop=True,
                                             skip_group_check=True)
                            bc_sb = bcp.tile([128, 512], f32)
                            nc.vector.tensor_copy(bc_sb, bc)
                            nc.vector.tensor_tensor(
                                ctxT[:, hl*2048 + ic*512: hl*2048 + (ic+1)*512],
                                cp, bc_sb, mybir.AluOpType.mult)

                # ---------------- Stage C: output projection ----------------
                with (
                    tc.tile_pool(name="psO", bufs=4, space="PSUM") as psO,
                    tc.tile_pool(name="outp", bufs=3) as outp,
                ):
                    for st in range(16):
                        for dok in range(4):
                            po = psO.tile([128, 512], f32)
                            for hl in range(8):
                                nc.tensor.matmul(
                                    po,
                                    ctxT[:, hl*2048 + st*128: hl*2048 + (st+1)*128],
                                    wout[:, hl*2048 + dok*512: hl*2048 + (dok+1)*512],
                                    start=(hl == 0), stop=(hl == 7))
                            ob_sb = outp.tile([128, 512], f32)
                            nc.vector.tensor_copy(ob_sb, po)
                            nc.sync.dma_start(
                                out_ap[st*128:(st+1)*128, dok*512:(dok+1)*512], ob_sb)

    nc.compile()
    return nc


def _prep_inputs(x, attn_mask, w_qkv, b_qkv, w_out, b_out):
    isd = INV_SQRT_DH
    x_imgs = []
    for b in range(4):
        x_imgs.append(np.ascontiguousarray(
            x[b].reshape(2048, 16, 128).transpose(2, 1, 0).reshape(128, 32768)
        ).astype(BF))

    jj = np.arange(128)[:, None]
    ii = np.arange(512)[None, :]
    mblocks = []
    for t in range(4):
        keep = attn_mask[ii, t*128 + jj] != 0
        mblocks.append(np.where(keep, 0.0, -1e30))
    masks = np.concatenate(mblocks, axis=1).astype(np.float32)

    ones_col = np.ones((128, 1), dtype=BF)
    ones_row = np.ones((1, 128), dtype=np.float32)

    per_g = []
    for g in range(2):
        rows = np.empty((2048, 2048), np.float32)
        bqk = np.empty((128, 16), np.float32)
        wv_rows = np.empty((1024, 2048), np.float32)
        bv = np.empty(1024, np.float32)
        for hl in range(8):
            h = 8*g + hl
            rows[(2*hl)*128:(2*hl+1)*128] = w_qkv[384*h: 384*h+128] * isd
            rows[(2*hl+1)*128:(2*hl+2)*128] = w_qkv[384*h+128: 384*h+256]
            bqk[:, 2*hl] = b_qkv[384*h: 384*h+128] * isd
            bqk[:, 2*hl+1] = b_qkv[384*h+128: 384*h+256]
            wv_rows[hl*128:(hl+1)*128] = w_qkv[384*h+256: 384*h+384]
            bv[hl*128:(hl+1)*128] = b_qkv[384*h+256: 384*h+384]
        wqk_img = np.ascontiguousarray(
            rows.reshape(16, 128, 16, 128).transpose(3, 0, 2, 1).reshape(128, 32768)
        ).astype(BF)
        wv_img = np.ascontiguousarray(
            wv_rows.reshape(1024, 16, 128).transpose(2, 1, 0).reshape(128, 16384)
        ).astype(BF)
        wout_img = np.ascontiguousarray(
            w_out[:, 1024*g: 1024*(g+1)].reshape(2048, 8, 128)
            .transpose(2, 1, 0).reshape(128, 16384)
        ).astype(BF)
        bias_v = np.ascontiguousarray(
            np.broadcast_to(bv[None, :], (128, 1024))).astype(np.float32)
        per_g.append((wqk_img, wv_img, wout_img, bqk, bias_v))

    in_maps = []
    for c in range(8):
        b, g = c // 2, c % 2
        wqk_img, wv_img, wout_img, bqk, bias_v = per_g[g]
        in_maps.append({
            "x_img": x_imgs[b],
            "wqk_img": wqk_img,
            "wv_img": wv_img,
            "wout_img": wout_img,
            "bqk": bqk,
            "bias_v": bias_v,
            "masks": masks,
            "ones_col": ones_col,
            "ones_row": ones_row,
        })
    return in_maps


def kernel(x, attn_mask, w_qkv, b_qkv, w_out, b_out):
    global LAST_EXEC_NS
    if "nc" not in _CACHE:
        _CACHE["nc"] = _build()
    nc = _CACHE["nc"]
    in_maps = _prep_inputs(
        np.asarray(x, np.float32), np.asarray(attn_mask),
        np.asarray(w_qkv, np.float32), np.asarray(b_qkv, np.float32),
        np.asarray(w_out, np.float32), np.asarray(b_out, np.float32))
    t0 = time.time()
    res = run_bass_kernel_spmd(nc, in_maps, list(range(8)), trace=False)
    t1 = time.time()
    LAST_EXEC_NS = res.exec_time_ns if res.exec_time_ns else int((t1 - t0) * 1e9)
    out = np.empty((4, 2048, 2048), np.float32)
    bo = np.asarray(b_out, np.float32)
    for b in range(4):
        out[b] = res.results[2*b]["out"] + res.results[2*b+1]["out"] + bo[None, :]
    return out


# revision 6
# speedup vs baseline: 257.9535x; 257.9535x over previous
import sys, os, time
sys.path.insert(0, '/opt/trn_rl_repo')
os.environ.setdefault("JAX_PLATFORMS", "")

import numpy as np
import ml_dtypes

import concourse.bass as bass
import concourse.bacc as bacc
import concourse.mybir as mybir
import concourse.tile as tile
from concourse import bass2jax

BF = ml_dtypes.bfloat16
B, S, D, H, DH = 4, 2048, 2048, 16, 128
INV_SQRT_DH = 1.0 / np.sqrt(128.0)

_CACHE = {}
LAST_EXEC_NS = None


def _build():
    nc = bacc.Bacc("TRN2", target_bir_lowering=False, debug=False, num_devices=8)
    f32, bf16 = mybir.dt.float32, mybir.dt.bfloat16
    x_ap = nc.dram_tensor("x_img", (128, 32768), bf16, kind="ExternalInput").ap()
    wqk_ap = nc.dram_tensor("wqk_img", (128, 32768), bf16, kind="ExternalInput").ap()
    wv_ap = nc.dram_tensor("wv_img", (128, 16384), bf16, kind="ExternalInput").ap()
    wout_ap = nc.dram_tensor("wout_img", (128, 16384), bf16, kind="ExternalInput").ap()
    bqk_ap = nc.dram_tensor("bqk", (128, 16), f32, kind="ExternalInput").ap()
    bv_ap = nc.dram_tensor("bias_v", (128, 1024), f32, kind="ExternalInput").ap()
    mask_ap = nc.dram_tensor("masks", (128, 2048), f32, kind="ExternalInput").ap()
    onc_ap = nc.dram_tensor("ones_col", (128, 1), bf16, kind="ExternalInput").ap()
    onr_ap = nc.dram_tensor("ones_row", (1, 128), f32, kind="ExternalInput").ap()
    out_ap = nc.dram_tensor("out", (2048, 2048), f32, kind="ExternalOutput").ap()

    with tile.TileContext(nc) as tc:
        with tc.tile_pool(name="persist", bufs=1) as pp:
            qk_sb = pp.tile([128, 32768], bf16)    # [dh, ob*2048+s], ob=2hl: Q_h^T, 2hl+1: K_h^T
            v_sb = pp.tile([128, 16384], bf16)     # [s%128, st*1024 + hl*128 + dh]
            bqk_sb = pp.tile([128, 16], f32)
            bv_sb = pp.tile([128, 1024], f32)
            ones_c = pp.tile([128, 1], bf16)
            ones_r = pp.tile([1, 128], f32)
            nc.sync.dma_start(bqk_sb, bqk_ap)
            nc.sync.dma_start(bv_sb, bv_ap)
            nc.sync.dma_start(ones_c, onc_ap)
            nc.sync.dma_start(ones_r, onr_ap)

            # ---------------- Stage A: QKV projections ----------------
            with tc.tile_pool(name="xTp", bufs=1) as xTp:
                xT = xTp.tile([128, 32768], bf16)  # [d%128, dt*2048 + s]
                for dt in range(16):
                    nc.sync.dma_start(xT[:, dt*2048:(dt+1)*2048],
                                      x_ap[:, dt*2048:(dt+1)*2048])

                # A-V first: v_sb = x @ Wv^T + bv   (psum[s, ov])
                with (
                    tc.tile_pool(name="wvp", bufs=1) as wvp,
                    tc.tile_pool(name="psV", bufs=4, space="PSUM") as psV,
                ):
                    for oc in range(2):
                        wv_oc = wvp.tile([128, 8192], bf16)  # [d%128, dt*512 + ov%512]
                        for dt in range(16):
                            nc.sync.dma_start(
                                wv_oc[:, dt*512:(dt+1)*512],
                                wv_ap[:, dt*1024 + oc*512: dt*1024 + (oc+1)*512])
                        for st in range(16):
                            ps = psV.tile([128, 512], f32)
                            for dt in range(16):
                                nc.tensor.matmul(
                                    ps,
                                    xT[:, dt*2048 + st*128: dt*2048 + (st+1)*128],
                                    wv_oc[:, dt*512:(dt+1)*512],
                                    start=(dt == 0), stop=(dt == 15))
                            nc.vector.tensor_tensor(
                                v_sb[:, st*1024 + oc*512: st*1024 + (oc+1)*512],
                                ps, bv_sb[:, oc*512:(oc+1)*512], mybir.AluOpType.add)

                # A-QK: qk_sb = Wqk @ x^T + b      (psum[o, s])
                with (
                    tc.tile_pool(name="wqkp", bufs=2) as wqkp,
                    tc.tile_pool(name="psQ", bufs=4, space="PSUM") as psQ,
                ):
                    for ob in range(16):
                        slab = wqkp.tile([128, 2048], bf16)  # [d%128, dt*128 + o]
                        nc.sync.dma_start(slab, wqk_ap[:, ob*2048:(ob+1)*2048])
                        for sc in range(4):
                            ps = psQ.tile([128, 512], f32)
                            for dt in range(16):
                                nc.tensor.matmul(
                                    ps,
                                    slab[:, dt*128:(dt+1)*128],
                                    xT[:, dt*2048 + sc*512: dt*2048 + (sc+1)*512],
                                    start=(dt == 0), stop=(dt == 15))
                            nc.vector.tensor_scalar_add(
                                qk_sb[:, ob*2048 + sc*512: ob*2048 + (sc+1)*512],
                                ps, bqk_sb[:, ob:ob+1])

            # ---------------- Stage B: attention ----------------
            with tc.tile_pool(name="bstat", bufs=1) as bstat:
                masks = bstat.tile([128, 2048], f32)
                ctxT = bstat.tile([128, 16384], bf16)  # [dh, hl*2048 + q]
                wout = bstat.tile([128, 16384], bf16)  # [dh, hl*2048 + od]
                nc.sync.dma_start(masks, mask_ap)
                for i in range(8):
                    nc.sync.dma_start(wout[:, i*2048:(i+1)*2048],
                                      wout_ap[:, i*2048:(i+1)*2048])

                with (
                    tc.tile_pool(name="expp", bufs=4) as expp,
                    tc.tile_pool(name="accp", bufs=2) as accp,
                    tc.tile_pool(name="accbfp", bufs=2) as accbfp,
                    tc.tile_pool(name="recp", bufs=2) as recp,
                    tc.tile_pool(name="bcp", bufs=2) as bcp,
                    tc.tile_pool(name="psS", bufs=4, space="PSUM") as psS,
                    tc.tile_pool(name="psC", bufs=2, space="PSUM") as psC,
                    tc.tile_pool(name="psD", bufs=1, space="PSUM") as psD,
                    tc.tile_pool(name="psB", bufs=1, space="PSUM") as psB,
                ):
                    for hl in range(8):
                        qb = (2*hl) * 2048
                        kb = (2*hl+1) * 2048
                        for ic in range(4):
                            njt = 4*ic + 4
                            acc = accp.tile([128, 512], f32)
                            acc_bf = accbfp.tile([128, 512], bf16)
                            cp = psC.tile([128, 512], f32)
                            pend = []

                            def flush_ctx():
                                jt_, ex_ = pend.pop(0)
                                nc.tensor.matmul(
                                    cp, v_sb[:, jt_*1024 + hl*128: jt_*1024 + (hl+1)*128],
                                    ex_, start=(jt_ == 0), stop=(jt_ == njt-1),
                                    skip_group_check=True)

                            for jt in range(njt):
                                sc = psS.tile([128, 512], f32)
                                nc.tensor.matmul(
                                    sc, qk_sb[:, kb + jt*128: kb + (jt+1)*128],
                                    qk_sb[:, qb + ic*512: qb + (ic+1)*512],
                                    start=True, stop=True, skip_group_check=True)
                                t = jt - 4*ic
                                if t >= 0:
                                    nc.vector.tensor_tensor(
                                        sc, sc, masks[:, t*512:(t+1)*512],
                                        mybir.AluOpType.add)
                                ex = expp.tile([128, 512], bf16)
                                nc.scalar.activation(ex, sc,
                                                     mybir.ActivationFunctionType.Exp)
                                if jt == 0:
                                    nc.vector.tensor_copy(acc, ex)
                                elif jt < njt - 1:
                                    nc.vector.tensor_tensor(acc, acc, ex,
                                                            mybir.AluOpType.add)
                                else:
                                    nc.vector.tensor_tensor(acc_bf, acc, ex,
                                                            mybir.AluOpType.add)
                                pend.append((jt, ex))
                                if len(pend) > 2:
                                    flush_ctx()
                            while pend:
                                flush_ctx()

                            dn = psD.tile([1, 512], f32)
                            nc.tensor.matmul(dn, ones_c, acc_bf, start=True, stop=True,
                                             skip_group_check=True)
                            rec = recp.tile([1, 512], f32)
                            nc.vector.reciprocal(rec, dn)
                            bc = psB.tile([128, 512], f32)
                            nc.tensor.matmul(bc, ones_r, rec, start=True, stop=True,
                                             skip_group_check=True)
                            bc_sb = bcp.tile([128, 512], f32)
                            nc.vector.tensor_copy(bc_sb, bc)
                            nc.vector.tensor_tensor(
                                ctxT[:, hl*2048 + ic*512: hl*2048 + (ic+1)*512],
                                cp, bc_sb, mybir.AluOpType.mult)

                # ---------------- Stage C: output projection ----------------
                with (
                    tc.tile_pool(name="psO", bufs=4, space="PSUM") as psO,
                    tc.tile_pool(name="outp", bufs=3) as outp,
                ):
                    for st in range(16):
                        for dok in range(4):
                            po = psO.tile([128, 512], f32)
                            for hl in range(8):
                                nc.tensor.matmul(
                                    po,
                                    ctxT[:, hl*2048 + st*128: hl*2048 + (st+1)*128],
                                    wout[:, hl*2048 + dok*512: hl*2048 + (dok+1)*512],
                                    start=(hl == 0), stop=(hl == 7))
                            ob_sb = outp.tile([128, 512], f32)
                            nc.vector.tensor_copy(ob_sb, po)
                            nc.sync.dma_start(
                                out_ap[st*128:(st+1)*128, dok*512:(dok+1)*512], ob_sb)

    nc.compile()
    return nc


def _prep_inputs(x, attn_mask, w_qkv, b_qkv, w_out, b_out):
    isd = INV_SQRT_DH
    x_imgs = []
    for b in range(4):
        x_imgs.append(np.ascontiguousarray(
            x[b].reshape(2048, 16, 128).transpose(2, 1, 0).reshape(128, 32768)
        ).astype(BF))

    jj = np.arange(128)[:, None]
    ii = np.arange(512)[None, :]
    mblocks = []
    for t in range(4):
        keep = attn_mask[ii, t*128 + jj] != 0
        mblocks.append(np.where(keep, 0.0, -1e30))
    masks = np.concatenate(mblocks, axis=1).astype(np.float32)

    ones_col = np.ones((128, 1), dtype=BF)
    ones_row = np.ones((1, 128), dtype=np.float32)

    per_g = []
    for g in range(2):
        rows = np.empty((2048, 2048), np.float32)
        bqk = np.empty((128, 16), np.float32)
        wv_rows = np.empty((1024, 2048), np.float32)
        bv = np.empty(1024, np.float32)
        for hl in range(8):
            h = 8*g + hl
            rows[(2*hl)*128:(2*hl+1)*128] = w_qkv[384*h: 384*h+128] * isd
            rows[(2*hl+1)*128:(2*hl+2)*128] = w_qkv[384*h+128: 384*h+256]
            bqk[:, 2*hl] = b_qkv[384*h: 384*h+128] * isd
            bqk[:, 2*hl+1] = b_qkv[384*h+128: 384*h+256]
            wv_rows[hl*128:(hl+1)*128] = w_qkv[384*h+256: 384*h+384]
            bv[hl*128:(hl+1)*128] = b_qkv[384*h+256: 384*h+384]
        wqk_img = np.ascontiguousarray(
            rows.reshape(16, 128, 16, 128).transpose(3, 0, 2, 1).reshape(128, 32768)
        ).astype(BF)
        wv_img = np.ascontiguousarray(
            wv_rows.reshape(1024, 16, 128).transpose(2, 1, 0).reshape(128, 16384)
        ).astype(BF)
        wout_img = np.ascontiguousarray(
            w_out[:, 1024*g: 1024*(g+1)].reshape(2048, 8, 128)
            .transpose(2, 1, 0).reshape(128, 16384)
        ).astype(BF)
        bias_v = np.ascontiguousarray(
            np.broadcast_to(bv[None, :], (128, 1024))).astype(np.float32)
        per_g.append((wqk_img, wv_img, wout_img, bqk, bias_v))

    in_maps = []
    for c in range(8):
        b, g = c // 2, c % 2
        wqk_img, wv_img, wout_img, bqk, bias_v = per_g[g]
        in_maps.append({
            "x_img": x_imgs[b],
            "wqk_img": wqk_img,
            "wv_img": wv_img,
            "wout_img": wout_img,
            "bqk": bqk,
            "bias_v": bias_v,
            "masks": masks,
            "ones_col": ones_col,
            "ones_row": ones_row,
        })
    return in_maps


N_CORES = 8


def _get_runner():
    """Build nc + jitted shard_map executable once per process."""
    if "runner" in _CACHE:
        return _CACHE["runner"]
    import jax
    from jax.sharding import Mesh, PartitionSpec
    from jax.experimental.shard_map import shard_map

    nc = _build()
    bass2jax.install_neuronx_cc_hook()

    partition_name = (nc.partition_id_tensor.name
                      if nc.partition_id_tensor else None)
    in_names, out_names, out_avals, zero_outs = [], [], [], []
    for alloc in nc.m.functions[0].allocations:
        if not isinstance(alloc, mybir.MemoryLocationSet):
            continue
        name = alloc.memorylocations[0].name
        if alloc.kind == "ExternalInput":
            if name != partition_name:
                in_names.append(name)
        elif alloc.kind == "ExternalOutput":
            out_names.append(name)
            shape = tuple(alloc.tensor_shape)
            dtype = mybir.dt.np(alloc.dtype)
            out_avals.append(jax.core.ShapedArray(shape, dtype))
            zero_outs.append(np.zeros(shape, dtype))
    n_params = len(in_names)
    n_outs = len(out_avals)
    all_names = in_names + out_names
    if partition_name is not None:
        all_names = all_names + [partition_name]

    def _body(*args):
        operands = list(args)
        if partition_name is not None:
            operands.append(bass2jax.partition_id_tensor())
        outs = bass2jax._bass_exec_p.bind(
            *operands,
            out_avals=tuple(out_avals),
            in_names=tuple(all_names),
            out_names=tuple(out_names),
            lowering_input_output_aliases=(),
            sim_require_finite=True,
            sim_require_nnan=True,
            nc=nc,
        )
        return tuple(outs)

    devices = jax.devices()[:N_CORES]
    mesh = Mesh(np.asarray(devices), ("core",))
    in_specs = (PartitionSpec("core"),) * (n_params + n_outs)
    out_specs = (PartitionSpec("core"),) * n_outs
    donate = tuple(range(n_params, n_params + n_outs))
    sharded = jax.jit(
        shard_map(_body, mesh=mesh, in_specs=in_specs, out_specs=out_specs,
                  check_rep=False),
        donate_argnums=donate, keep_unused=True)
    sharded_nodonate = jax.jit(
        shard_map(_body, mesh=mesh, in_specs=in_specs, out_specs=out_specs,
                  check_rep=False),
        keep_unused=True)
    _CACHE["runner"] = (sharded, sharded_nodonate, in_names, out_names,
                        zero_outs, mesh)
    return _CACHE["runner"]


def _concat_inputs(in_maps, in_names):
    return [np.concatenate([m[name] for m in in_maps], axis=0)
            for name in in_names]


def kernel(x, attn_mask, w_qkv, b_qkv, w_out, b_out):
    global LAST_EXEC_NS
    sharded, _, in_names, out_names, zero_outs, _ = _get_runner()
    in_maps = _prep_inputs(
        np.asarray(x, np.float32), np.asarray(attn_mask),
        np.asarray(w_qkv, np.float32), np.asarray(b_qkv, np.float32),
        np.asarray(w_out, np.float32), np.asarray(b_out, np.float32))
    concat_in = _concat_inputs(in_maps, in_names)
    concat_zeros = [np.zeros((N_CORES * z.shape[0], *z.shape[1:]), z.dtype)
                    for z in zero_outs]
    t0 = time.time()
    out_arrs = sharded(*concat_in, *concat_zeros)
    res = np.asarray(out_arrs[0]).reshape(N_CORES, 2048, 2048)
    t1 = time.time()
    LAST_EXEC_NS = int((t1 - t0) * 1e9)
    bo = np.asarray(b_out, np.float32)
    out = np.empty((4, 2048, 2048), np.float32)
    for b in range(4):
        out[b] = res[2*b] + res[2*b+1] + bo[None, :]
    return out
